# revision 1
# baseline (speedup 1.0000x reference)
"""Trainium2 Bass kernel for nn_LocalLocalContrastiveLoss.

Math (see reference): z = z_t.reshape(N=4096, D=256); logits row i =
[sim(i, ·) with self masked, z@memQ.T] / T; lse_i = logsumexp(row);
per_pair_i = lse_i - sim(i, i+1)/T; loss = mean over valid anchors
(i % L != L-1), n_pairs = 4080.  va_values is unused (faithful to ref).

Distribution: 8 cores, each handles 512 anchors (4 blocks of 128).
Negatives (all of z + memory queue) are replicated. To keep one
core-agnostic NEFF, each core's copy of z^T is ROTATED so its own 512
anchor columns come first; then the self-diagonal / +1 diagonal sit at
fixed block positions identical on every core.

Per anchor-block b (128 anchors) the 20480 logit columns are processed
in 10 chunks of 2048: matmul (K=256 split in 2) -> PSUM [128,2048],
DVE reduce_max (negated), ACT exp(bias=-max) with accumulator -> chunk
sums; chunk (max, sum) pairs are combined at the end into lse.
pos-sims come from the +1-shifted diagonal of chunk 0 via an eye mask.
Host sums valid per-pair losses.
"""

import os
import sys
from contextlib import ExitStack

import numpy as np

sys.path.insert(0, "/opt/trn_rl_repo")

import concourse.bass as bass  # noqa: E402
import concourse.bacc as bacc  # noqa: E402
import concourse.tile as tile  # noqa: E402
from concourse import mybir  # noqa: E402
from concourse.bass_utils import run_bass_kernel_spmd  # noqa: E402

B, L, D = 16, 256, 256
N = B * L            # 4096 anchors
K = 16384            # memory queue
INV_T = 1.0 / 0.07
NCORES = 8
APC = N // NCORES    # anchors per core = 512
NB = APC // 128      # anchor blocks per core = 4
CH = 2048            # chunk width (4 PSUM banks)
NCOLS = N + K        # 20480
NCH = NCOLS // CH    # 10 chunks (2 from z, 8 from memq)
SUB = 512            # matmul moving free dim (fp32 max)
F32 = mybir.dt.float32


def _build_nc(skip_c0=False, skip_combine=False) -> bass.Bass:
    nc = bacc.Bacc("TRN2", target_bir_lowering=False, debug=False)

    anch = nc.dram_tensor("anch", [2, 128, APC], F32, kind="ExternalInput")
    zrot = nc.dram_tensor("zrot", [2, 128, N], F32, kind="ExternalInput")
    memq = nc.dram_tensor("memq", [2, 128, K], F32, kind="ExternalInput")
    eyen = nc.dram_tensor("eyen", [128, 128], F32, kind="ExternalInput")
    eyep = nc.dram_tensor("eyep", [128, 128], F32, kind="ExternalInput")
    lse_out = nc.dram_tensor("lse_out", [128, NB], F32, kind="ExternalOutput")
    pos_out = nc.dram_tensor("pos_out", [128, NB], F32, kind="ExternalOutput")

    with tile.TileContext(nc) as tc, ExitStack() as ctx:
        consts = ctx.enter_context(tc.tile_pool(name="consts", bufs=1))
        rhsp = ctx.enter_context(tc.tile_pool(name="rhs", bufs=2))
        psum = ctx.enter_context(tc.tile_pool(name="psum", bufs=2, space="PSUM"))
        stats = ctx.enter_context(tc.tile_pool(name="stats", bufs=1))
        small = ctx.enter_context(tc.tile_pool(name="small", bufs=4))

        # Constants / stationary weights
        anch_sb = [consts.tile([128, APC], F32, tag=f"anch{k}", name=f"anch{k}") for k in range(2)]
        for k in range(2):
            nc.sync.dma_start(anch_sb[k][:], anch[k])
        eyen_sb = consts.tile([128, 128], F32, tag="eyen", name="eyen_sb")
        nc.sync.dma_start(eyen_sb[:], eyen[:])
        eyep_sb = consts.tile([128, 128], F32, tag="eyep", name="eyep_sb")
        nc.sync.dma_start(eyep_sb[:], eyep[:])

        nm_all = stats.tile([128, NB * NCH], F32, tag="nm", name="nm_all")   # negated chunk maxes
        s_all = stats.tile([128, NB * NCH], F32, tag="s", name="s_all")     # chunk exp-sums
        lse_sb = stats.tile([128, NB], F32, tag="lse", name="lse_sb")
        pos_sb = stats.tile([128, NB], F32, tag="pos", name="pos_sb")

        for c in range(NCH):
            rt = [rhsp.tile([128, CH], F32, tag=f"rt{k}", name=f"rt{k}") for k in range(2)]
            for k in range(2):
                if c < 2:
                    src = zrot[k, :, c * CH:(c + 1) * CH]
                else:
                    src = memq[k, :, (c - 2) * CH:(c - 1) * CH]
                nc.sync.dma_start(rt[k][:], src)

            for b in range(NB):
                pt = psum.tile([128, CH], F32, tag="pt", name="pt")
                for k in range(2):
                    lhsT = anch_sb[k][:, b * 128:(b + 1) * 128]
                    for s in range(CH // SUB):
                        nc.tensor.matmul(
                            pt[:, s * SUB:(s + 1) * SUB],
                            lhsT,
                            rt[k][:, s * SUB:(s + 1) * SUB],
                            start=(k == 0),
                            stop=(k == 1),
                        )
                bc = b * NCH + c
                if c == 0 and not skip_c0:
                    # mask self-sim on the block diagonal: -= 1e30 * eye
                    diag = pt[:, b * 128:(b + 1) * 128]
                    nc.vector.tensor_sub(diag, diag, eyen_sb[:])
                    # pos-sim: +1-shifted diagonal, via eye mask + row-sum.
                    # (tensor_tensor_reduce with a PSUM operand dies on HW,
                    # so stage the window through SBUF first.)
                    win_sb = small.tile([128, 128], F32, tag="winsb", name="win_sb")
                    nc.vector.tensor_copy(win_sb[:], pt[:, b * 128 + 1:b * 128 + 129])
                    posw = small.tile([128, 128], F32, tag="posw", name="posw")
                    nc.vector.tensor_mul(posw[:], win_sb[:], eyep_sb[:])
                    nc.vector.reduce_sum(
                        out=pos_sb[:, b:b + 1], in_=posw[:],
                        axis=mybir.AxisListType.X,
                    )
                nc.vector.reduce_max(
                    out=nm_all[:, bc:bc + 1], in_=pt[:], axis=mybir.AxisListType.X,
                    negate=True,
                )
                nc.scalar.activation(
                    out=pt[:], in_=pt[:], func=mybir.ActivationFunctionType.Exp,
                    bias=nm_all[:, bc:bc + 1], scale=1.0,
                    accum_out=s_all[:, bc:bc + 1],
                )

        # Combine chunks -> lse per block.  Grouped by op to avoid ACT
        # table-set thrash (all Exp, then all Log).
        if skip_combine:
            nc.vector.tensor_copy(lse_sb[:], nm_all[:, 0:NB])
            if skip_c0:
                nc.vector.tensor_copy(pos_sb[:], s_all[:, 0:NB])
        else:
            nM = [small.tile([128, 1], F32, tag=f"nM{b}", name=f"nM{b}") for b in range(NB)]
            eb = [small.tile([128, NCH], F32, tag=f"eb{b}", name=f"eb{b}") for b in range(NB)]
            Sb = [small.tile([128, 1], F32, tag=f"Sb{b}", name=f"Sb{b}") for b in range(NB)]
            lgb = [small.tile([128, 1], F32, tag=f"lgb{b}", name=f"lgb{b}") for b in range(NB)]
            for b in range(NB):
                nc.vector.tensor_reduce(
                    out=nM[b][:], in_=nm_all[:, b * NCH:(b + 1) * NCH],
                    axis=mybir.AxisListType.X, op=mybir.AluOpType.min,
                )
            for b in range(NB):
                # exp(-nm_c + nM) = exp(m_c - M)
                nc.scalar.activation(
                    out=eb[b][:], in_=nm_all[:, b * NCH:(b + 1) * NCH],
                    func=mybir.ActivationFunctionType.Exp,
                    bias=nM[b][:], scale=-1.0,
                )
            for b in range(NB):
                sw = small.tile([128, NCH], F32, tag=f"sw{b}", name=f"sw{b}")
                nc.vector.tensor_mul(sw[:], s_all[:, b * NCH:(b + 1) * NCH], eb[b][:])
                nc.vector.reduce_sum(
                    out=Sb[b][:], in_=sw[:], axis=mybir.AxisListType.X,
                )
            for b in range(NB):
                nc.scalar.activation(
                    out=lgb[b][:], in_=Sb[b][:],
                    func=mybir.ActivationFunctionType.Ln,
                )
            for b in range(NB):
                # lse = log(S) + M = log(S) - nM
                nc.vector.tensor_sub(lse_sb[:, b:b + 1], lgb[b][:], nM[b][:])

        nc.sync.dma_start(lse_out[:], lse_sb[:])
        nc.sync.dma_start(pos_out[:], pos_sb[:])

    nc.compile()
    return nc


_NC_CACHE = None


def _get_nc():
    global _NC_CACHE
    if _NC_CACHE is None:
        import os as _os
        _NC_CACHE = _build_nc(skip_c0=bool(_os.environ.get('SKIP_C0')), skip_combine=bool(_os.environ.get('SKIP_COMBINE')))
    return _NC_CACHE


def make_in_maps(z_t: np.ndarray, memory_queue: np.ndarray):
    z = np.ascontiguousarray(z_t.reshape(N, D)).astype(np.float32)
    zT = np.ascontiguousarray(z.T)                      # [D, N]
    memT = np.ascontiguousarray(memory_queue.astype(np.float32).T)  # [D, K]
    memT = memT.reshape(2, 128, K)
    eyen = (np.eye(128, dtype=np.float32) * 1e30)
    eyep = np.eye(128, dtype=np.float32)

    in_maps = []
    for r in range(NCORES):
        zr = np.roll(zT, -APC * r, axis=1)              # own cols first
        anch = np.ascontiguousarray(zr[:, :APC]) * np.float32(INV_T)
        in_maps.append({
            "anch": np.ascontiguousarray(anch.reshape(2, 128, APC)),
            "zrot": np.ascontiguousarray(zr.reshape(2, 128, N)),
            "memq": memT,
            "eyen": eyen,
            "eyep": eyep,
        })
    return in_maps


def combine_outputs(results) -> np.ndarray:
    # results[r]["lse_out"/"pos_out"]: [128, NB]; global anchor
    # g = 512*r + 128*b + p  ->  per_pair[g] = lse - pos
    pp = np.empty(N, dtype=np.float64)
    for r in range(NCORES):
        lse = np.asarray(results[r]["lse_out"], dtype=np.float64)
        pos = np.asarray(results[r]["pos_out"], dtype=np.float64)
        for b in range(NB):
            g0 = APC * r + 128 * b
            pp[g0:g0 + 128] = lse[:, b] - pos[:, b]
    idx = np.arange(N - 1)
    valid = (idx % L) != (L - 1)
    loss = pp[:N - 1][valid].sum() / valid.sum()
    return np.float32(loss)


def kernel(z_t, va_values=None, memory_queue=None, _trace=False):
    nc = _get_nc()
    in_maps = make_in_maps(z_t, memory_queue)
    res = run_bass_kernel_spmd(
        nc, in_maps, core_ids=list(range(NCORES)), trace=_trace,
    )
    out = combine_outputs(res.results)
    if _trace:
        kernel.last_result = res
    return out


if __name__ == "__main__":
    rng = np.random.default_rng(0)
    z_t = rng.standard_normal((B, L, D), dtype=np.float32)
    mq = rng.standard_normal((K, D), dtype=np.float32)
    va = rng.random((B, L, 2), dtype=np.float32)
    loss = kernel(z_t, va, mq)
    print("device loss:", loss)
    # numpy reference check
    z = z_t.reshape(N, D).astype(np.float64)
    sim = (z @ z.T) * INV_T
    msim = (z @ mq.astype(np.float64).T) * INV_T
    np.fill_diagonal(sim, -np.inf)
    logits = np.concatenate([sim, msim], axis=1)
    m = logits.max(axis=1, keepdims=True)
    lse = np.log(np.exp(logits - m).sum(axis=1)) + m[:, 0]
    pos = np.array([(z[i] @ z[i + 1]) * INV_T for i in range(N - 1)])
    ppz = -pos + lse[:-1]
    vald = (np.arange(N - 1) % L) != (L - 1)
    ref = ppz[vald].sum() / vald.sum()
    print("numpy  loss:", ref, " rel err:", abs(loss - ref) / abs(ref))



# revision 2
# speedup vs baseline: 1.8337x; 1.8337x over previous
"""Trainium2 Bass kernel for nn_LocalLocalContrastiveLoss.

Math (see reference): z = z_t.reshape(N=4096, D=256); logits row i =
[sim(i, ·) with self masked, z@memQ.T] / T; lse_i = logsumexp(row);
per_pair_i = lse_i - sim(i, i+1)/T; loss = mean over valid anchors
(i % L != L-1), n_pairs = 4080.  va_values is unused (faithful to ref).

Distribution: 8 cores, each handles 512 anchors (4 blocks of 128).
Negatives (all of z + memory queue) are replicated. To keep one
core-agnostic NEFF, each core's copy of z^T is ROTATED so its own 512
anchor columns come first; then the self-diagonal / +1 diagonal sit at
fixed block positions identical on every core.

v2 changes vs v1:
- All matmul inputs are bf16 (host-converted): 1 cyc/row on PE instead
  of 4 for fp32 (PE ~273us -> ~68us), and input DMA halves.
- No on-device chunk combine: the kernel exports per-(block,chunk)
  (negated max, exp-sum) stats plus pos-sims; the host merges them in
  fp64 (drops the ACT Ln table load + serial tail).

Per anchor-block b (128 anchors) the 20480 logit columns are processed
in 10 chunks of 2048: matmul (K=256 split in 2) -> PSUM [128,2048],
DVE reduce_max (negated), ACT exp(bias=-max) with accumulator -> chunk
sums. pos-sims come from the +1-shifted diagonal of chunk 0 via an eye
mask. Host reduces chunk stats to lse and sums valid per-pair losses.
"""

import sys
from contextlib import ExitStack

import ml_dtypes
import numpy as np

sys.path.insert(0, "/opt/trn_rl_repo")

import concourse.bass as bass  # noqa: E402
import concourse.bacc as bacc  # noqa: E402
import concourse.tile as tile  # noqa: E402
from concourse import mybir  # noqa: E402
from concourse.bass_utils import run_bass_kernel_spmd  # noqa: E402

B, L, D = 16, 256, 256
N = B * L            # 4096 anchors
K = 16384            # memory queue
INV_T = 1.0 / 0.07
NCORES = 8
APC = N // NCORES    # anchors per core = 512
NB = APC // 128      # anchor blocks per core = 4
CH = 2048            # chunk width (4 PSUM banks)
NCOLS = N + K        # 20480
NCH = NCOLS // CH    # 10 chunks (2 from z, 8 from memq)
SUB = 512            # matmul moving free dim (one PSUM bank)
F32 = mybir.dt.float32
BF16 = mybir.dt.bfloat16


def _build_nc() -> bass.Bass:
    nc = bacc.Bacc("TRN2", target_bir_lowering=False, debug=False)

    anch = nc.dram_tensor("anch", [2, 128, APC], BF16, kind="ExternalInput")
    zrot = nc.dram_tensor("zrot", [2, 128, N], BF16, kind="ExternalInput")
    memq = nc.dram_tensor("memq", [2, 128, K], BF16, kind="ExternalInput")
    eyen = nc.dram_tensor("eyen", [128, 128], F32, kind="ExternalInput")
    eyep = nc.dram_tensor("eyep", [128, 128], F32, kind="ExternalInput")
    nm_out = nc.dram_tensor("nm_out", [128, NB * NCH], F32, kind="ExternalOutput")
    s_out = nc.dram_tensor("s_out", [128, NB * NCH], F32, kind="ExternalOutput")
    pos_out = nc.dram_tensor("pos_out", [128, NB], F32, kind="ExternalOutput")

    with tile.TileContext(nc) as tc, ExitStack() as ctx:
        consts = ctx.enter_context(tc.tile_pool(name="consts", bufs=1))
        rhsp = ctx.enter_context(tc.tile_pool(name="rhs", bufs=3))
        psum = ctx.enter_context(tc.tile_pool(name="psum", bufs=2, space="PSUM"))
        stats = ctx.enter_context(tc.tile_pool(name="stats", bufs=1))
        small = ctx.enter_context(tc.tile_pool(name="small", bufs=4))

        # Constants / stationary weights
        anch_sb = [consts.tile([128, APC], BF16, tag=f"anch{k}", name=f"anch{k}") for k in range(2)]
        for k in range(2):
            nc.sync.dma_start(anch_sb[k][:], anch[k])
        eyen_sb = consts.tile([128, 128], F32, tag="eyen", name="eyen_sb")
        nc.sync.dma_start(eyen_sb[:], eyen[:])
        eyep_sb = consts.tile([128, 128], F32, tag="eyep", name="eyep_sb")
        nc.sync.dma_start(eyep_sb[:], eyep[:])

        nm_all = stats.tile([128, NB * NCH], F32, tag="nm", name="nm_all")   # negated chunk maxes
        s_all = stats.tile([128, NB * NCH], F32, tag="s", name="s_all")     # chunk exp-sums
        pos_sb = stats.tile([128, NB], F32, tag="pos", name="pos_sb")

        for c in range(NCH):
            rt = [rhsp.tile([128, CH], BF16, tag=f"rt{k}", name=f"rt{k}") for k in range(2)]
            for k in range(2):
                if c < 2:
                    src = zrot[k, :, c * CH:(c + 1) * CH]
                else:
                    src = memq[k, :, (c - 2) * CH:(c - 1) * CH]
                nc.sync.dma_start(rt[k][:], src)

            for b in range(NB):
                pt = psum.tile([128, CH], F32, tag="pt", name="pt")
                for k in range(2):
                    lhsT = anch_sb[k][:, b * 128:(b + 1) * 128]
                    for s in range(CH // SUB):
                        nc.tensor.matmul(
                            pt[:, s * SUB:(s + 1) * SUB],
                            lhsT,
                            rt[k][:, s * SUB:(s + 1) * SUB],
                            start=(k == 0),
                            stop=(k == 1),
                        )
                bc = b * NCH + c
                if c == 0:
                    # mask self-sim on the block diagonal: -= 1e30 * eye
                    diag = pt[:, b * 128:(b + 1) * 128]
                    nc.vector.tensor_sub(diag, diag, eyen_sb[:])
                    # pos-sim: +1-shifted diagonal, via eye mask + row-sum.
                    # (tensor_tensor_reduce with a PSUM operand dies on HW,
                    # so stage the window through SBUF first; the copy runs
                    # on ACT, which has slack - DVE is the bottleneck.)
                    win_sb = small.tile([128, 128], F32, tag="winsb", name="win_sb")
                    nc.scalar.copy(win_sb[:], pt[:, b * 128 + 1:b * 128 + 129])
                    posw = small.tile([128, 128], F32, tag="posw", name="posw")
                    nc.vector.tensor_mul(posw[:], win_sb[:], eyep_sb[:])
                    nc.vector.reduce_sum(
                        out=pos_sb[:, b:b + 1], in_=posw[:],
                        axis=mybir.AxisListType.X,
                    )
                nc.vector.reduce_max(
                    out=nm_all[:, bc:bc + 1], in_=pt[:], axis=mybir.AxisListType.X,
                    negate=True,
                )
                nc.scalar.activation(
                    out=pt[:], in_=pt[:], func=mybir.ActivationFunctionType.Exp,
                    bias=nm_all[:, bc:bc + 1], scale=1.0,
                    accum_out=s_all[:, bc:bc + 1],
                )

        nc.sync.dma_start(nm_out[:], nm_all[:])
        nc.sync.dma_start(s_out[:], s_all[:])
        nc.sync.dma_start(pos_out[:], pos_sb[:])

    nc.compile()
    return nc


_NC_CACHE = None


def _get_nc():
    global _NC_CACHE
    if _NC_CACHE is None:
        _NC_CACHE = _build_nc()
    return _NC_CACHE


def make_in_maps(z_t: np.ndarray, memory_queue: np.ndarray):
    z = np.ascontiguousarray(z_t.reshape(N, D)).astype(np.float32)
    zT = np.ascontiguousarray(z.T)                      # [D, N]
    memT = np.ascontiguousarray(memory_queue.astype(np.float32).T)  # [D, K]
    memT = memT.reshape(2, 128, K).astype(ml_dtypes.bfloat16)
    eyen = (np.eye(128, dtype=np.float32) * 1e30)
    eyep = np.eye(128, dtype=np.float32)

    in_maps = []
    for r in range(NCORES):
        zr = np.roll(zT, -APC * r, axis=1)              # own cols first
        anch = (zr[:, :APC] * np.float32(INV_T)).astype(ml_dtypes.bfloat16)
        in_maps.append({
            "anch": np.ascontiguousarray(anch.reshape(2, 128, APC)),
            "zrot": np.ascontiguousarray(zr.astype(ml_dtypes.bfloat16).reshape(2, 128, N)),
            "memq": memT,
            "eyen": eyen,
            "eyep": eyep,
        })
    return in_maps


def combine_outputs(results) -> np.ndarray:
    # results[r]["nm_out"/"s_out"]: [128, NB*NCH]; ["pos_out"]: [128, NB].
    # global anchor g = 512*r + 128*b + p.
    # lse = M + log(sum_c S_c * exp(m_c - M)),  m_c = -nm_c, M = max_c m_c
    pp = np.empty(N, dtype=np.float64)
    for r in range(NCORES):
        m = -np.asarray(results[r]["nm_out"], dtype=np.float64)  # [128, NB*NCH]
        S = np.asarray(results[r]["s_out"], dtype=np.float64)
        pos = np.asarray(results[r]["pos_out"], dtype=np.float64)
        m = m.reshape(128, NB, NCH)
        S = S.reshape(128, NB, NCH)
        M = m.max(axis=2)                                        # [128, NB]
        lse = M + np.log(np.sum(S * np.exp(m - M[:, :, None]), axis=2))
        for b in range(NB):
            g0 = APC * r + 128 * b
            pp[g0:g0 + 128] = lse[:, b] - pos[:, b]
    idx = np.arange(N - 1)
    valid = (idx % L) != (L - 1)
    loss = pp[:N - 1][valid].sum() / valid.sum()
    return np.float32(loss)


def kernel(z_t, va_values=None, memory_queue=None, _trace=False):
    nc = _get_nc()
    in_maps = make_in_maps(z_t, memory_queue)
    res = run_bass_kernel_spmd(
        nc, in_maps, core_ids=list(range(NCORES)), trace=_trace,
    )
    out = combine_outputs(res.results)
    if _trace:
        kernel.last_result = res
    return out


if __name__ == "__main__":
    rng = np.random.default_rng(0)
    z_t = rng.standard_normal((B, L, D), dtype=np.float32)
    mq = rng.standard_normal((K, D), dtype=np.float32)
    va = rng.random((B, L, 2), dtype=np.float32)
    loss = kernel(z_t, va, mq)
    print("device loss:", loss)
    # numpy reference check
    z = z_t.reshape(N, D).astype(np.float64)
    sim = (z @ z.T) * INV_T
    msim = (z @ mq.astype(np.float64).T) * INV_T
    np.fill_diagonal(sim, -np.inf)
    logits = np.concatenate([sim, msim], axis=1)
    m = logits.max(axis=1, keepdims=True)
    lse = np.log(np.exp(logits - m).sum(axis=1)) + m[:, 0]
    pos = np.array([(z[i] @ z[i + 1]) * INV_T for i in range(N - 1)])
    ppz = -pos + lse[:-1]
    vald = (np.arange(N - 1) % L) != (L - 1)
    ref = ppz[vald].sum() / vald.sum()
    print("numpy  loss:", ref, " rel err:", abs(loss - ref) / abs(ref))


# revision 3
# speedup vs baseline: 2.1952x; 1.1971x over previous
"""Trainium2 Bass kernel for nn_LocalLocalContrastiveLoss.

Math (see reference): z = z_t.reshape(N=4096, D=256); logits row i =
[sim(i, ·) with self masked, z@memQ.T] / T; lse_i = logsumexp(row);
per_pair_i = lse_i - sim(i, i+1)/T; loss = mean over valid anchors
(i % L != L-1), n_pairs = 4080.  va_values is unused (faithful to ref).

Distribution: 8 cores, each handles 512 anchors (4 blocks of 128).
Negatives (all of z + memory queue) are replicated. To keep one
core-agnostic NEFF, each core's copy of z^T is ROTATED so its own 512
anchor columns come first; then the self-diagonal sits at fixed block
positions identical on every core.

v3 design (after trace analysis of v2):
- bf16 matmul inputs (host-converted): 1 cyc/row on PE, half DMA.
- The whole 20480-col rhs stays RESIDENT in SBUF (40 bf16 tiles of
  [128,1024]; ~80KB/partition) - DMA'd once, reused by all 4 blocks.
- 1024-wide PSUM regions x 4 in flight (the full 8 banks). v2 used
  2x2048 which serialized the MM->reduce_max->exp chain (each engine
  only ~53% busy); 4 regions let DVE/ACT/PE run concurrently.
- Per (chunk, block): 4 matmuls (2 K-halves x 2x512) -> DVE reduce_max
  (negated) -> ACT exp(bias=-max) with accum_out -> per-chunk sums.
- pos-sims (z_i . z_{i+1}) and the final logsumexp merge + masked mean
  are done on host in fp64 (tiny: ~4096x256 MACs + 4096x80 merges).
"""

import sys
from contextlib import ExitStack

import ml_dtypes
import numpy as np

sys.path.insert(0, "/opt/trn_rl_repo")

import concourse.bass as bass  # noqa: E402
import concourse.bacc as bacc  # noqa: E402
import concourse.tile as tile  # noqa: E402
from concourse import mybir  # noqa: E402
from concourse.bass_utils import run_bass_kernel_spmd  # noqa: E402

B, L, D = 16, 256, 256
N = B * L            # 4096 anchors
K = 16384            # memory queue
INV_T = 1.0 / 0.07
NCORES = 8
APC = N // NCORES    # anchors per core = 512
NB = APC // 128      # anchor blocks per core = 4
CH = 1024            # chunk width (2 PSUM banks)
NCOLS = N + K        # 20480
NCH = NCOLS // CH    # 20 chunks (4 from z, 16 from memq)
SUB = 512            # matmul moving free dim (one PSUM bank)
F32 = mybir.dt.float32
BF16 = mybir.dt.bfloat16


def _build_nc() -> bass.Bass:
    nc = bacc.Bacc("TRN2", target_bir_lowering=False, debug=False)

    anch = nc.dram_tensor("anch", [2, 128, APC], BF16, kind="ExternalInput")
    zrot = nc.dram_tensor("zrot", [2, 128, N], BF16, kind="ExternalInput")
    memq = nc.dram_tensor("memq", [2, 128, K], BF16, kind="ExternalInput")
    eyen = nc.dram_tensor("eyen", [128, 128], F32, kind="ExternalInput")
    nm_out = nc.dram_tensor("nm_out", [128, NB * NCH], F32, kind="ExternalOutput")
    s_out = nc.dram_tensor("s_out", [128, NB * NCH], F32, kind="ExternalOutput")

    with tile.TileContext(nc) as tc, ExitStack() as ctx:
        consts = ctx.enter_context(tc.tile_pool(name="consts", bufs=1))
        psum = ctx.enter_context(tc.tile_pool(name="psum", bufs=4, space="PSUM"))
        stats = ctx.enter_context(tc.tile_pool(name="stats", bufs=1))

        # Stationary anchors (lhsT) + self-mask constant.
        anch_sb = [consts.tile([128, APC], BF16, tag=f"anch{k}", name=f"anch{k}") for k in range(2)]
        for k in range(2):
            nc.sync.dma_start(anch_sb[k][:], anch[k])
        eyen_sb = consts.tile([128, 128], F32, tag="eyen", name="eyen_sb")
        nc.sync.dma_start(eyen_sb[:], eyen[:])

        # Whole rhs resident in SBUF: 20 chunks x 2 K-halves, DMA'd once.
        rt = [[consts.tile([128, CH], BF16, tag=f"rt{c}_{k}", name=f"rt{c}_{k}")
               for k in range(2)] for c in range(NCH)]
        for c in range(NCH):
            for k in range(2):
                if c < N // CH:
                    src = zrot[k, :, c * CH:(c + 1) * CH]
                else:
                    src = memq[k, :, (c - N // CH) * CH:(c + 1 - N // CH) * CH]
                nc.sync.dma_start(rt[c][k][:], src)

        nm_all = stats.tile([128, NB * NCH], F32, tag="nm", name="nm_all")   # negated chunk maxes
        s_all = stats.tile([128, NB * NCH], F32, tag="s", name="s_all")     # chunk exp-sums

        for c in range(NCH):
            for b in range(NB):
                pt = psum.tile([128, CH], F32, tag="pt", name="pt")
                for k in range(2):
                    lhsT = anch_sb[k][:, b * 128:(b + 1) * 128]
                    for s in range(CH // SUB):
                        nc.tensor.matmul(
                            pt[:, s * SUB:(s + 1) * SUB],
                            lhsT,
                            rt[c][k][:, s * SUB:(s + 1) * SUB],
                            start=(k == 0),
                            stop=(k == 1),
                        )
                bc = b * NCH + c
                if c == 0:
                    # mask self-sim on the block diagonal: -= 1e30 * eye
                    diag = pt[:, b * 128:(b + 1) * 128]
                    nc.vector.tensor_sub(diag, diag, eyen_sb[:])
                nc.vector.reduce_max(
                    out=nm_all[:, bc:bc + 1], in_=pt[:], axis=mybir.AxisListType.X,
                    negate=True,
                )
                nc.scalar.activation(
                    out=pt[:], in_=pt[:], func=mybir.ActivationFunctionType.Exp,
                    bias=nm_all[:, bc:bc + 1], scale=1.0,
                    accum_out=s_all[:, bc:bc + 1],
                )

        nc.sync.dma_start(nm_out[:], nm_all[:])
        nc.sync.dma_start(s_out[:], s_all[:])

    nc.compile()
    return nc


_NC_CACHE = None


def _get_nc():
    global _NC_CACHE
    if _NC_CACHE is None:
        _NC_CACHE = _build_nc()
    return _NC_CACHE


def make_in_maps(z_t: np.ndarray, memory_queue: np.ndarray):
    z = np.ascontiguousarray(z_t.reshape(N, D)).astype(np.float32)
    zT = np.ascontiguousarray(z.T)                      # [D, N]
    memT = np.ascontiguousarray(memory_queue.astype(np.float32).T)  # [D, K]
    memT = memT.reshape(2, 128, K).astype(ml_dtypes.bfloat16)
    eyen = (np.eye(128, dtype=np.float32) * 1e30)

    in_maps = []
    for r in range(NCORES):
        zr = np.roll(zT, -APC * r, axis=1)              # own cols first
        anch = (zr[:, :APC] * np.float32(INV_T)).astype(ml_dtypes.bfloat16)
        in_maps.append({
            "anch": np.ascontiguousarray(anch.reshape(2, 128, APC)),
            "zrot": np.ascontiguousarray(zr.astype(ml_dtypes.bfloat16).reshape(2, 128, N)),
            "memq": memT,
            "eyen": eyen,
        })
    return in_maps


def combine_outputs(results, z: np.ndarray) -> np.ndarray:
    # results[r]["nm_out"/"s_out"]: [128, NB*NCH]; anchor g = 512r+128b+p.
    # lse = M + log(sum_c S_c * exp(m_c - M)),  m_c = -nm_c, M = max_c m_c
    lse_all = np.empty(N, dtype=np.float64)
    for r in range(NCORES):
        m = -np.asarray(results[r]["nm_out"], dtype=np.float64).reshape(128, NB, NCH)
        S = np.asarray(results[r]["s_out"], dtype=np.float64).reshape(128, NB, NCH)
        M = m.max(axis=2)                                        # [128, NB]
        lse = M + np.log(np.sum(S * np.exp(m - M[:, :, None]), axis=2))
        for b in range(NB):
            g0 = APC * r + 128 * b
            lse_all[g0:g0 + 128] = lse[:, b]
    z64 = z.astype(np.float64)
    pos = np.einsum("ij,ij->i", z64[:-1], z64[1:]) * INV_T       # [N-1]
    pp = lse_all[:N - 1] - pos
    valid = (np.arange(N - 1) % L) != (L - 1)
    loss = pp[valid].sum() / valid.sum()
    return np.float32(loss)


def kernel(z_t, va_values=None, memory_queue=None, _trace=False):
    nc = _get_nc()
    in_maps = make_in_maps(z_t, memory_queue)
    res = run_bass_kernel_spmd(
        nc, in_maps, core_ids=list(range(NCORES)), trace=_trace,
    )
    out = combine_outputs(res.results, z_t.reshape(N, D))
    if _trace:
        kernel.last_result = res
    return out


if __name__ == "__main__":
    rng = np.random.default_rng(0)
    z_t = rng.standard_normal((B, L, D), dtype=np.float32)
    mq = rng.standard_normal((K, D), dtype=np.float32)
    va = rng.random((B, L, 2), dtype=np.float32)
    loss = kernel(z_t, va, mq)
    print("device loss:", loss)
    # numpy reference check
    z = z_t.reshape(N, D).astype(np.float64)
    sim = (z @ z.T) * INV_T
    msim = (z @ mq.astype(np.float64).T) * INV_T
    np.fill_diagonal(sim, -np.inf)
    logits = np.concatenate([sim, msim], axis=1)
    m = logits.max(axis=1, keepdims=True)
    lse = np.log(np.exp(logits - m).sum(axis=1)) + m[:, 0]
    pos = np.array([(z[i] @ z[i + 1]) * INV_T for i in range(N - 1)])
    ppz = -pos + lse[:-1]
    vald = (np.arange(N - 1) % L) != (L - 1)
    ref = ppz[vald].sum() / vald.sum()
    print("numpy  loss:", ref, " rel err:", abs(loss - ref) / abs(ref))


# revision 6
# speedup vs baseline: 2.5846x; 1.1774x over previous
"""Trainium2 Bass kernel for nn_LocalLocalContrastiveLoss.

Math (see reference): z = z_t.reshape(N=4096, D=256); logits row i =
[sim(i, ·) with self masked, z@memQ.T] / T; lse_i = logsumexp(row);
per_pair_i = lse_i - sim(i, i+1)/T; loss = mean over valid anchors
(i % L != L-1), n_pairs = 4080.  va_values is unused (faithful to ref).

Distribution: 8 cores, each handles 512 anchors (4 blocks of 128).
Negatives (all of z + memory queue) are replicated. To keep one
core-agnostic NEFF, each core's copy of z^T is ROTATED so its own 512
anchor columns come first; then the self-diagonal sits at fixed block
positions identical on every core.

v3 design (after trace analysis of v2):
- bf16 matmul inputs (host-converted): 1 cyc/row on PE, half DMA.
- The whole 20480-col rhs stays RESIDENT in SBUF (40 bf16 tiles of
  [128,1024]; ~80KB/partition) - DMA'd once, reused by all 4 blocks.
- 1024-wide PSUM regions x 4 in flight (the full 8 banks). v2 used
  2x2048 which serialized the MM->reduce_max->exp chain (each engine
  only ~53% busy); 4 regions let DVE/ACT/PE run concurrently.
- Per (chunk, block): 4 matmuls (2 K-halves x 2x512) -> DVE reduce_max
  (negated) -> ACT exp(bias=-max) with accum_out -> per-chunk sums.
- pos-sims (z_i . z_{i+1}) and the final logsumexp merge + masked mean
  are done on host in fp64 (tiny: ~4096x256 MACs + 4096x80 merges).
"""

import sys
from contextlib import ExitStack

import ml_dtypes
import numpy as np

sys.path.insert(0, "/opt/trn_rl_repo")

import concourse.bass as bass  # noqa: E402
import concourse.bacc as bacc  # noqa: E402
import concourse.tile as tile  # noqa: E402
from concourse import mybir  # noqa: E402
from concourse.bass_utils import run_bass_kernel_spmd  # noqa: E402

B, L, D = 16, 256, 256
N = B * L            # 4096 anchors
K = 16384            # memory queue
INV_T = 1.0 / 0.07
NCORES = 8
APC = N // NCORES    # anchors per core = 512
NB = APC // 128      # anchor blocks per core = 4
CH = 1024            # chunk width (2 PSUM banks)
NCOLS = N + K        # 20480
NCH = NCOLS // CH    # 20 chunks (4 from z, 16 from memq)
SUB = 512            # matmul moving free dim (one PSUM bank)
F32 = mybir.dt.float32
BF16 = mybir.dt.bfloat16


def _build_nc() -> bass.Bass:
    nc = bacc.Bacc("TRN2", target_bir_lowering=False, debug=False)

    # K-halves share the middle dim so each chunk is ONE DMA (host lays
    # the data out as [128 partitions, 2 K-halves, cols]).
    anch = nc.dram_tensor("anch", [128, 2, APC], BF16, kind="ExternalInput")
    zrot = nc.dram_tensor("zrot", [128, 2, N], BF16, kind="ExternalInput")
    memq = nc.dram_tensor("memq", [128, 2, K], BF16, kind="ExternalInput")
    eyen = nc.dram_tensor("eyen", [128, 128], F32, kind="ExternalInput")
    stats_out = nc.dram_tensor(
        "stats_out", [128, 2, NB * NCH], F32, kind="ExternalOutput"
    )

    with tile.TileContext(nc) as tc, ExitStack() as ctx:
        consts = ctx.enter_context(tc.tile_pool(name="consts", bufs=1))
        psum = ctx.enter_context(tc.tile_pool(name="psum", bufs=4, space="PSUM"))
        stats = ctx.enter_context(tc.tile_pool(name="stats", bufs=1))

        # Stationary anchors (lhsT) + self-mask constant.
        anch_sb = consts.tile([128, 2, APC], BF16, tag="anch", name="anch_sb")
        nc.sync.dma_start(anch_sb[:], anch[:])
        eyen_sb = consts.tile([128, 128], F32, tag="eyen", name="eyen_sb")
        nc.sync.dma_start(eyen_sb[:], eyen[:])

        # Whole rhs resident in SBUF: 20 chunks x [128, 2, CH], DMA'd once.
        rt = [consts.tile([128, 2, CH], BF16, tag=f"rt{c}", name=f"rt{c}")
              for c in range(NCH)]
        for c in range(NCH):
            if c < N // CH:
                src = zrot[:, :, c * CH:(c + 1) * CH]
            else:
                src = memq[:, :, (c - N // CH) * CH:(c + 1 - N // CH) * CH]
            nc.sync.dma_start(rt[c][:], src)

        # stats[:, 0, bc] = negated chunk max; stats[:, 1, bc] = exp-sum
        st = stats.tile([128, 2, NB * NCH], F32, tag="st", name="st")

        for c in range(NCH):
            for b in range(NB):
                pt = psum.tile([128, CH], F32, tag="pt", name="pt")
                for k in range(2):
                    lhsT = anch_sb[:, k, b * 128:(b + 1) * 128]
                    for s in range(CH // SUB):
                        nc.tensor.matmul(
                            pt[:, s * SUB:(s + 1) * SUB],
                            lhsT,
                            rt[c][:, k, s * SUB:(s + 1) * SUB],
                            start=(k == 0),
                            stop=(k == 1),
                        )
                bc = b * NCH + c
                if c == 0:
                    # mask self-sim on the block diagonal: -= 1e30 * eye
                    diag = pt[:, b * 128:(b + 1) * 128]
                    nc.vector.tensor_sub(diag, diag, eyen_sb[:])
                nc.vector.reduce_max(
                    out=st[:, 0, bc:bc + 1], in_=pt[:], axis=mybir.AxisListType.X,
                    negate=True,
                )
                nc.scalar.activation(
                    out=pt[:], in_=pt[:], func=mybir.ActivationFunctionType.Exp,
                    bias=st[:, 0, bc:bc + 1], scale=1.0,
                    accum_out=st[:, 1, bc:bc + 1],
                )

        nc.sync.dma_start(stats_out[:], st[:])

    nc.compile()
    return nc


_NC_CACHE = None


def _get_nc():
    global _NC_CACHE
    if _NC_CACHE is None:
        _NC_CACHE = _build_nc()
    return _NC_CACHE


def _k_mid(a: np.ndarray) -> np.ndarray:
    # [D, cols] -> [128 partitions, 2 K-halves, cols]
    return np.ascontiguousarray(a.reshape(2, 128, a.shape[1]).transpose(1, 0, 2))


def make_in_maps(z_t: np.ndarray, memory_queue: np.ndarray):
    z = np.ascontiguousarray(z_t.reshape(N, D)).astype(np.float32)
    zT = np.ascontiguousarray(z.T)                      # [D, N]
    memT = np.ascontiguousarray(memory_queue.astype(np.float32).T)  # [D, K]
    memT = _k_mid(memT.astype(ml_dtypes.bfloat16))
    eyen = (np.eye(128, dtype=np.float32) * 1e30)

    in_maps = []
    for r in range(NCORES):
        zr = np.roll(zT, -APC * r, axis=1)              # own cols first
        anch = (zr[:, :APC] * np.float32(INV_T)).astype(ml_dtypes.bfloat16)
        in_maps.append({
            "anch": _k_mid(anch),
            "zrot": _k_mid(zr.astype(ml_dtypes.bfloat16)),
            "memq": memT,
            "eyen": eyen,
        })
    return in_maps


def combine_outputs(results, z: np.ndarray) -> np.ndarray:
    # results[r]["nm_out"/"s_out"]: [128, NB*NCH]; anchor g = 512r+128b+p.
    # lse = M + log(sum_c S_c * exp(m_c - M)),  m_c = -nm_c, M = max_c m_c
    lse_all = np.empty(N, dtype=np.float64)
    for r in range(NCORES):
        stats = np.asarray(results[r]["stats_out"], dtype=np.float64)
        m = -stats[:, 0, :].reshape(128, NB, NCH)
        S = stats[:, 1, :].reshape(128, NB, NCH)
        M = m.max(axis=2)                                        # [128, NB]
        lse = M + np.log(np.sum(S * np.exp(m - M[:, :, None]), axis=2))
        for b in range(NB):
            g0 = APC * r + 128 * b
            lse_all[g0:g0 + 128] = lse[:, b]
    z64 = z.astype(np.float64)
    pos = np.einsum("ij,ij->i", z64[:-1], z64[1:]) * INV_T       # [N-1]
    pp = lse_all[:N - 1] - pos
    valid = (np.arange(N - 1) % L) != (L - 1)
    loss = pp[valid].sum() / valid.sum()
    return np.float32(loss)


def kernel(z_t, va_values=None, memory_queue=None, _trace=False):
    nc = _get_nc()
    in_maps = make_in_maps(z_t, memory_queue)
    res = run_bass_kernel_spmd(
        nc, in_maps, core_ids=list(range(NCORES)), trace=_trace,
    )
    out = combine_outputs(res.results, z_t.reshape(N, D))
    if _trace:
        kernel.last_result = res
    return out


if __name__ == "__main__":
    rng = np.random.default_rng(0)
    z_t = rng.standard_normal((B, L, D), dtype=np.float32)
    mq = rng.standard_normal((K, D), dtype=np.float32)
    va = rng.random((B, L, 2), dtype=np.float32)
    loss = kernel(z_t, va, mq)
    print("device loss:", loss)
    # numpy reference check
    z = z_t.reshape(N, D).astype(np.float64)
    sim = (z @ z.T) * INV_T
    msim = (z @ mq.astype(np.float64).T) * INV_T
    np.fill_diagonal(sim, -np.inf)
    logits = np.concatenate([sim, msim], axis=1)
    m = logits.max(axis=1, keepdims=True)
    lse = np.log(np.exp(logits - m).sum(axis=1)) + m[:, 0]
    pos = np.array([(z[i] @ z[i + 1]) * INV_T for i in range(N - 1)])
    ppz = -pos + lse[:-1]
    vald = (np.arange(N - 1) % L) != (L - 1)
    ref = ppz[vald].sum() / vald.sum()
    print("numpy  loss:", ref, " rel err:", abs(loss - ref) / abs(ref))
